# revision 1
# baseline (speedup 1.0000x reference)
"""Co_GCN (2-layer mixed-adjacency GCN) on 8 Trainium2 NeuronCores.

Math:
  h   = relu(A1 @ (x@W1) + b1),  A1 = sum_k softmax(pi1)[k] * adj[k]
  hm  = dropout_mask * h / 0.7   (fixed jax key 42)
  out = A2 @ (hm@W2) + b2,       A2 = sum_k pi2[k] * adj[k]

Sharding (8 cores):
  Layer 1: row-sharded  — core d owns output rows  B_d (1024 rows), reads adj[:, B_d, :].
  Layer 2: col-sharded  — core d owns contraction cols B_d, reads adj[:, :, B_d],
           uses its LOCAL support2 block, emits a full-size partial outT [16, 8192].
  Host: out = (sum_d partial_d).T + b2.   (No on-device collective needed.)

On-core dataflow (both layers): adj tiles are loaded natural [i-part, j-free],
transposed 128x128 on the PE (transpose-mode ~85ns/tile, value-exact), evacuated
PSUM->SBUF on DVE/ACT alternately, then contracted j-on-partitions with the
support matrix as the stationary operand.  The k-mixture weights (softmax(pi1),
pi2) are folded into pre-scaled copies of the support matrices (w1cat/w2cat),
and the 4 k-terms accumulate in PSUM via matmul start/stop groups.
"""

import numpy as np

N = 8192
K = 4
F_IN = 128
F_HID = 64
F_OUT = 16
NCORES = 8
R = N // NCORES          # 1024 rows per core
P = 128
JC = N // P              # 64 j-chunks of 128
DROPOUT_P = 0.3

_prog_cache = {}


def _dropout_mask_T():
    """(mask/keep).T as [F_HID, N] fp32, reproducing the reference's fixed key."""
    import jax
    cpu = jax.devices("cpu")[0]
    with jax.default_device(cpu):
        keep = jax.random.bernoulli(jax.random.key(42), 1.0 - DROPOUT_P, (N, F_HID))
        m = np.asarray(keep, dtype=np.float32) / np.float32(1.0 - DROPOUT_P)
    return np.ascontiguousarray(m.T)  # [64, 8192]


def _build_program():
    import concourse.bacc as bacc
    import concourse.mybir as mybir
    from concourse import tile

    dt = mybir.dt.float32
    nc = bacc.Bacc("TRN2", target_bir_lowering=False, debug=False,
                   num_devices=NCORES)

    adjb = nc.dram_tensor("adjb", [K, R, N], dt, kind="ExternalInput")
    adjc = nc.dram_tensor("adjc", [K, N, R], dt, kind="ExternalInput")
    xT = nc.dram_tensor("xT", [F_IN, N], dt, kind="ExternalInput")
    w1cat = nc.dram_tensor("w1cat", [F_IN, K * F_HID], dt, kind="ExternalInput")
    w2cat = nc.dram_tensor("w2cat", [F_HID, K * F_OUT], dt, kind="ExternalInput")
    maskTb = nc.dram_tensor("maskTb", [F_HID, R], dt, kind="ExternalInput")
    b1c = nc.dram_tensor("b1c", [F_HID, 1], dt, kind="ExternalInput")
    idn = nc.dram_tensor("idn", [P, P], dt, kind="ExternalInput")
    outT = nc.dram_tensor("outT", [F_OUT, N], dt, kind="ExternalOutput")

    mult = mybir.AluOpType.mult
    Relu = mybir.ActivationFunctionType.Relu

    with tile.TileContext(nc) as tc:
        with (
            tc.tile_pool(name="const", bufs=1) as cpool,
            tc.tile_pool(name="big", bufs=1) as bigpool,
            tc.tile_pool(name="slab", bufs=2) as slabpool,
            tc.tile_pool(name="ats", bufs=8) as atspool,
            tc.tile_pool(name="small", bufs=2) as smallpool,
            tc.tile_pool(name="mm", bufs=3, space="PSUM") as mmpool,
            tc.tile_pool(name="at", bufs=5, space="PSUM") as atpool,
        ):
            ident = cpool.tile([P, P], dt)
            w1s = cpool.tile([F_IN, K * F_HID], dt)
            w2s = cpool.tile([F_HID, K * F_OUT], dt)
            mks = cpool.tile([F_HID, R], dt)
            b1s = cpool.tile([F_HID, 1], dt)
            nc.sync.dma_start(ident[:], idn[:])
            nc.sync.dma_start(w1s[:], w1cat[:])
            nc.sync.dma_start(w2s[:], w2cat[:])
            nc.sync.dma_start(mks[:], maskTb[:])
            nc.sync.dma_start(b1s[:], b1c[:])

            xTs = bigpool.tile([F_IN, N], dt)
            nc.sync.dma_start(xTs[:], xT[:])

            # ---- support1, pre-scaled per k:  s1k[j, k*64+f] = x@(pi1n[k] W1) ----
            s1k = bigpool.tile([P, JC * K * F_HID], dt)        # [128, 16384]
            for jc in range(JC):
                sp = mmpool.tile([P, K * F_HID], dt, tag="mm", name=f"s1p{jc % 3}")
                nc.tensor.matmul(sp[:], xTs[:, jc * P:(jc + 1) * P], w1s[:],
                                 start=True, stop=True)
                nc.vector.tensor_copy(s1k[:, jc * 256:(jc + 1) * 256], sp[:])

            # ---- layer 1 main:  hT[f, i] += adjT[j, i-part] per (jc, k) ----
            hp0 = mmpool.tile([F_HID, 512], dt, tag="mm", name="hp0")
            hp1 = mmpool.tile([F_HID, 512], dt, tag="mm", name="hp1")
            hps = [hp0, hp1]
            ei = 0
            for jc in range(JC):
                slab = slabpool.tile([P, K * 8, P], dt, tag="slab", name=f"sl{jc % 2}")
                src = adjb[:, :, jc * P:(jc + 1) * P].rearrange(
                    "k (a p) j -> p (k a) j", p=P)
                nc.sync.dma_start(slab[:], src)
                for k in range(K):
                    for ih in range(2):
                        at = atpool.tile([P, 512], dt, tag="at",
                                         name=f"at{(jc * 8 + k * 2 + ih) % 5}")
                        for a in range(4):
                            nc.tensor.transpose(
                                at[:, a * P:(a + 1) * P],
                                slab[:, k * 8 + ih * 4 + a, :], ident[:])
                        ats = atspool.tile([P, 512], dt, tag="ats",
                                           name=f"ats{ei % 8}")
                        if ei % 2 == 0:
                            nc.vector.tensor_copy(ats[:], at[:])
                        else:
                            nc.scalar.copy(ats[:], at[:])
                        ei += 1
                        nc.tensor.matmul(
                            hps[ih][:],
                            s1k[:, jc * 256 + k * F_HID: jc * 256 + (k + 1) * F_HID],
                            ats[:],
                            start=(jc == 0 and k == 0), stop=(jc == JC - 1 and k == K - 1))

            # ---- h -> relu(+b1) -> *mask -> hmT [64, 1024] ----
            hmT = cpool.tile([F_HID, R], dt)
            for ih in range(2):
                nc.scalar.activation(hmT[:, ih * 512:(ih + 1) * 512], hps[ih][:],
                                     Relu, bias=b1s[:, 0:1])
            nc.vector.scalar_tensor_tensor(hmT[:], hmT[:], 1.0, mks[:], mult, mult)

            # ---- support2, pre-scaled per k: s2k[jloc, c*64 + k*16+g] ----
            s2k = cpool.tile([P, 8 * K * F_OUT], dt)           # [128, 512]
            for c in range(8):
                sp2 = mmpool.tile([P, K * F_OUT], dt, tag="mm", name=f"s2p{c % 3}")
                nc.tensor.matmul(sp2[:], hmT[:, c * P:(c + 1) * P], w2s[:],
                                 start=True, stop=True)
                nc.vector.tensor_copy(s2k[:, c * 64:(c + 1) * 64], sp2[:])

            # ---- layer 2: partial outT[g, i] over local j in B_d ----
            for istrip in range(16):
                op = mmpool.tile([F_OUT, 512], dt, tag="mm", name=f"op{istrip % 3}")
                for k in range(K):
                    slab2 = slabpool.tile([P, 4, N // 8], dt, tag="slab",
                                          name=f"sl2{(istrip * K + k) % 2}")
                    src2 = adjc[k, istrip * 512:(istrip + 1) * 512, :].rearrange(
                        "(a p) j -> p a j", p=P)
                    nc.sync.dma_start(slab2[:], src2)
                    for c in range(8):
                        at2 = atpool.tile([P, 512], dt, tag="at",
                                          name=f"at2{(istrip * 32 + k * 8 + c) % 5}")
                        for a in range(4):
                            nc.tensor.transpose(
                                at2[:, a * P:(a + 1) * P],
                                slab2[:, a, c * P:(c + 1) * P], ident[:])
                        ats2 = atspool.tile([P, 512], dt, tag="ats",
                                            name=f"ats2{ei % 8}")
                        if ei % 2 == 0:
                            nc.vector.tensor_copy(ats2[:], at2[:])
                        else:
                            nc.scalar.copy(ats2[:], at2[:])
                        ei += 1
                        nc.tensor.matmul(
                            op[:],
                            s2k[:, c * 64 + k * F_OUT: c * 64 + (k + 1) * F_OUT],
                            ats2[:],
                            start=(k == 0 and c == 0), stop=(k == K - 1 and c == 7))
                ot = smallpool.tile([F_OUT, 512], dt, tag="ot", name=f"ot{istrip % 2}")
                nc.scalar.copy(ot[:], op[:])
                nc.sync.dma_start(outT[:, istrip * 512:(istrip + 1) * 512], ot[:])

    nc.compile()
    return nc


def _run(inputs, trace=False):
    from concourse.bass_utils import run_bass_kernel_spmd

    adj = np.ascontiguousarray(inputs["adj"], dtype=np.float32)
    x = np.ascontiguousarray(inputs["x"], dtype=np.float32)
    W1 = np.asarray(inputs["W1"], dtype=np.float32)
    b1 = np.asarray(inputs["b1"], dtype=np.float32)
    W2 = np.asarray(inputs["W2"], dtype=np.float32)
    b2 = np.asarray(inputs["b2"], dtype=np.float32)
    pi1 = np.asarray(inputs["pi1"], dtype=np.float64)
    pi2 = np.asarray(inputs["pi2"], dtype=np.float32)

    e = np.exp(pi1 - pi1.max())
    pi1n = (e / e.sum()).astype(np.float32)

    w1cat = np.concatenate([pi1n[k] * W1 for k in range(K)], axis=1)  # [128, 256]
    w2cat = np.concatenate([pi2[k] * W2 for k in range(K)], axis=1)   # [64, 64]
    xT = np.ascontiguousarray(x.T)
    maskT = _dropout_mask_T()                                         # [64, 8192]
    idn = np.eye(P, dtype=np.float32)

    if "prog" not in _prog_cache:
        _prog_cache["prog"] = _build_program()
    nc = _prog_cache["prog"]

    in_maps = []
    for d in range(NCORES):
        r0, r1 = d * R, (d + 1) * R
        in_maps.append({
            "adjb": np.ascontiguousarray(adj[:, r0:r1, :]),
            "adjc": np.ascontiguousarray(adj[:, :, r0:r1]),
            "xT": xT,
            "w1cat": w1cat,
            "w2cat": w2cat,
            "maskTb": np.ascontiguousarray(maskT[:, r0:r1]),
            "b1c": b1.reshape(F_HID, 1),
            "idn": idn,
        })

    res = run_bass_kernel_spmd(nc, in_maps, core_ids=list(range(NCORES)),
                               trace=trace)
    acc = np.zeros((F_OUT, N), dtype=np.float64)
    for d in range(NCORES):
        acc += res.results[d]["outT"]
    out = acc.T.astype(np.float32) + b2[None, :]
    return out, res


def kernel(**inputs) -> np.ndarray:
    out, _ = _run(inputs, trace=False)
    return out


# revision 2
# speedup vs baseline: 1.6731x; 1.6731x over previous
"""Co_GCN (2-layer mixed-adjacency GCN) on 8 Trainium2 NeuronCores.

Math:
  h   = relu(A1 @ (x@W1) + b1),  A1 = sum_k softmax(pi1)[k] * adj[k]
  hm  = dropout_mask * h / 0.7   (fixed jax key 42)
  out = A2 @ (hm@W2) + b2,       A2 = sum_k pi2[k] * adj[k]

Sharding (8 cores):
  Layer 1: row-sharded  — core d owns output rows  B_d (1024), reads adj[:, B_d, :].
  Layer 2: col-sharded  — core d owns contraction cols B_d, reads adj[:, :, B_d],
           uses its LOCAL support2 block, emits a full-size partial outT [16, 8192].
  Host: out = (sum_d partial_d).T + b2.   (No on-device collective needed.)

Per-core dataflow: adj streams in as fp16 (SWDGE cast-DMA, 4KB contiguous bursts).
Each 128x128 chunk is transposed-and-k-mixed on the PE in one step: a regular
matmul with a pi-scaled fp16 identity as the moving operand computes
pi_k * chunk^T and accumulates the 4 k-slices in PSUM (a-outer/k-inner order —
accumulation groups within one PSUM tile must not interleave).  The mixed
transposed [128j, 512i] block is evacuated to fp16 SBUF (DVE/ACT alternating)
and contracted j-on-partitions against the unscaled fp16 support matrix.
"""

import numpy as np

N = 8192
K = 4
F_IN = 128
F_HID = 64
F_OUT = 16
NCORES = 8
R = N // NCORES          # 1024 rows per core
P = 128
DROPOUT_P = 0.3

_prog_cache = {}


def _dropout_mask_T():
    """(mask/keep).T as [F_HID, N] fp32, reproducing the reference's fixed key."""
    import jax
    cpu = jax.devices("cpu")[0]
    with jax.default_device(cpu):
        keep = jax.random.bernoulli(jax.random.key(42), 1.0 - DROPOUT_P, (N, F_HID))
        m = np.asarray(keep, dtype=np.float32) / np.float32(1.0 - DROPOUT_P)
    return np.ascontiguousarray(m.T)  # [64, 8192]


def _build_program():
    import concourse.bacc as bacc
    import concourse.mybir as mybir
    from concourse import tile

    f32 = mybir.dt.float32
    f16 = mybir.dt.float16
    nc = bacc.Bacc("TRN2", target_bir_lowering=False, debug=False,
                   num_devices=NCORES)

    adjb = nc.dram_tensor("adjb", [K, R, N], f32, kind="ExternalInput")
    adjc = nc.dram_tensor("adjc", [K, N, R], f32, kind="ExternalInput")
    xT = nc.dram_tensor("xT", [F_IN, N], f32, kind="ExternalInput")
    w1d = nc.dram_tensor("w1d", [F_IN, F_HID], f32, kind="ExternalInput")
    idn1d = nc.dram_tensor("idn1d", [P, K * P], f16, kind="ExternalInput")
    idn2d = nc.dram_tensor("idn2d", [P, K * P], f16, kind="ExternalInput")
    w2d = nc.dram_tensor("w2d", [F_HID, F_OUT], f32, kind="ExternalInput")
    maskTb = nc.dram_tensor("maskTb", [F_HID, R], f32, kind="ExternalInput")
    b1c = nc.dram_tensor("b1c", [F_HID, 1], f32, kind="ExternalInput")
    outT = nc.dram_tensor("outT", [F_OUT, N], f32, kind="ExternalOutput")

    mult = mybir.AluOpType.mult
    Relu = mybir.ActivationFunctionType.Relu

    with tile.TileContext(nc) as tc:
        with (
            tc.tile_pool(name="const", bufs=1) as cpool,
            tc.tile_pool(name="slab", bufs=5) as slabpool,
            tc.tile_pool(name="ats", bufs=8) as atspool,
            tc.tile_pool(name="small", bufs=2) as smallpool,
            tc.tile_pool(name="mm", bufs=3, space="PSUM") as mmpool,
            tc.tile_pool(name="at", bufs=5, space="PSUM") as atpool,
        ):
            idn1 = cpool.tile([P, K * P], f16)
            idn2 = cpool.tile([P, K * P], f16)
            w1s = cpool.tile([F_IN, F_HID], f32)
            w2s = cpool.tile([F_HID, F_OUT], f32)
            mks = cpool.tile([F_HID, R], f32)
            b1s = cpool.tile([F_HID, 1], f32)
            xTs = cpool.tile([F_IN, N], f32)
            nc.sync.dma_start(idn1[:], idn1d[:])
            nc.sync.dma_start(idn2[:], idn2d[:])
            nc.sync.dma_start(w1s[:], w1d[:])
            nc.sync.dma_start(w2s[:], w2d[:])
            nc.sync.dma_start(mks[:], maskTb[:])
            nc.sync.dma_start(b1s[:], b1c[:])
            nc.sync.dma_start(xTs[:], xT[:])

            # ---- support1 (unscaled):  s1[j, f] = (x@W1)[j, f], fp16, j on partitions
            s1 = cpool.tile([P, 64 * F_HID], f16)              # [128, 4096]
            for jg in range(64):
                sp = mmpool.tile([P, F_HID], f32, tag="mm", name=f"s1p{jg % 3}")
                nc.tensor.matmul(sp[:], xTs[:, jg * P:(jg + 1) * P], w1s[:],
                                 start=True, stop=True)
                nc.vector.tensor_copy(s1[:, jg * F_HID:(jg + 1) * F_HID], sp[:])

            # ---- layer 1:  hT[f, i] = sum_{k,j} pi1n[k] adj[k,i,j] s1[j, f] ----
            hp0 = mmpool.tile([F_HID, 512], f32, tag="mm", name="hp0")
            hp1 = mmpool.tile([F_HID, 512], f32, tag="mm", name="hp1")
            hps = [hp0, hp1]
            ei = 0
            for jh in range(8):
                slabs = []
                for k in range(K):
                    sl = slabpool.tile([P, 8, 1024], f16, tag="slab",
                                       name=f"sl{(jh * K + k) % 5}")
                    nc.gpsimd.dma_start(
                        sl[:],
                        adjb[k, :, jh * 1024:(jh + 1) * 1024].rearrange(
                            "(a p) j -> p a j", p=P))
                    slabs.append(sl)
                for jc in range(8):
                    jg = jh * 8 + jc
                    for ih in range(2):
                        at = atpool.tile([P, 512], f32, tag="at",
                                         name=f"at{ei % 5}")
                        for a in range(4):
                            for k in range(K):
                                nc.tensor.matmul(
                                    at[:, a * P:(a + 1) * P],
                                    slabs[k][:, ih * 4 + a, jc * P:(jc + 1) * P],
                                    idn1[:, k * P:(k + 1) * P],
                                    start=(k == 0), stop=(k == K - 1))
                        ats = atspool.tile([P, 512], f16, tag="ats",
                                           name=f"ats{ei % 8}")
                        if ei % 2 == 0:
                            nc.vector.tensor_copy(ats[:], at[:])
                        else:
                            nc.scalar.copy(ats[:], at[:])
                        ei += 1
                        nc.tensor.matmul(
                            hps[ih][:],
                            s1[:, jg * F_HID:(jg + 1) * F_HID],
                            ats[:],
                            start=(jg == 0), stop=(jg == 63))

            # ---- h -> relu(+b1) -> *mask -> hmT [64, 1024] fp32 ----
            hmT = cpool.tile([F_HID, R], f32)
            for ih in range(2):
                nc.scalar.activation(hmT[:, ih * 512:(ih + 1) * 512], hps[ih][:],
                                     Relu, bias=b1s[:, 0:1])
            nc.vector.scalar_tensor_tensor(hmT[:], hmT[:], 1.0, mks[:], mult, mult)

            # ---- support2 (unscaled): s2[jloc, g] fp16, jloc on partitions ----
            s2 = cpool.tile([P, 8 * F_OUT], f16)               # [128, 128]
            for c in range(8):
                sp2 = mmpool.tile([P, F_OUT], f32, tag="mm", name=f"s2p{c % 3}")
                nc.tensor.matmul(sp2[:], hmT[:, c * P:(c + 1) * P], w2s[:],
                                 start=True, stop=True)
                nc.vector.tensor_copy(s2[:, c * F_OUT:(c + 1) * F_OUT], sp2[:])

            # ---- layer 2: partial outT[g, i] = sum_{k, jloc} pi2[k] adj[k,i,j] s2 ----
            for ist in range(8):
                slabs2 = []
                for k in range(K):
                    sl2 = slabpool.tile([P, 8, 1024], f16, tag="slab",
                                        name=f"sl2{(ist * K + k) % 5}")
                    nc.gpsimd.dma_start(
                        sl2[:],
                        adjc[k, ist * 1024:(ist + 1) * 1024, :].rearrange(
                            "(a p) j -> p a j", p=P))
                    slabs2.append(sl2)
                for ih in range(2):
                    op = mmpool.tile([F_OUT, 512], f32, tag="mm",
                                     name=f"op{(ist * 2 + ih) % 3}")
                    for jc in range(8):
                        at2 = atpool.tile([P, 512], f32, tag="at",
                                          name=f"at2{ei % 5}")
                        for a in range(4):
                            for k in range(K):
                                nc.tensor.matmul(
                                    at2[:, a * P:(a + 1) * P],
                                    slabs2[k][:, ih * 4 + a, jc * P:(jc + 1) * P],
                                    idn2[:, k * P:(k + 1) * P],
                                    start=(k == 0), stop=(k == K - 1))
                        ats2 = atspool.tile([P, 512], f16, tag="ats",
                                            name=f"ats2{ei % 8}")
                        if ei % 2 == 0:
                            nc.vector.tensor_copy(ats2[:], at2[:])
                        else:
                            nc.scalar.copy(ats2[:], at2[:])
                        ei += 1
                        nc.tensor.matmul(
                            op[:],
                            s2[:, jc * F_OUT:(jc + 1) * F_OUT],
                            ats2[:],
                            start=(jc == 0), stop=(jc == 7))
                    ot = smallpool.tile([F_OUT, 512], f32, tag="ot",
                                        name=f"ot{(ist * 2 + ih) % 2}")
                    nc.scalar.copy(ot[:], op[:])
                    nc.sync.dma_start(
                        outT[:, ist * 1024 + ih * 512: ist * 1024 + (ih + 1) * 512],
                        ot[:])

    nc.compile()
    return nc


def _run(inputs, trace=False):
    from concourse.bass_utils import run_bass_kernel_spmd

    adj = np.ascontiguousarray(inputs["adj"], dtype=np.float32)
    x = np.ascontiguousarray(inputs["x"], dtype=np.float32)
    W1 = np.asarray(inputs["W1"], dtype=np.float32)
    b1 = np.asarray(inputs["b1"], dtype=np.float32)
    W2 = np.asarray(inputs["W2"], dtype=np.float32)
    b2 = np.asarray(inputs["b2"], dtype=np.float32)
    pi1 = np.asarray(inputs["pi1"], dtype=np.float64)
    pi2 = np.asarray(inputs["pi2"], dtype=np.float32)

    e = np.exp(pi1 - pi1.max())
    pi1n = (e / e.sum()).astype(np.float32)

    eye = np.eye(P)
    idn1 = np.concatenate([eye * pi1n[k] for k in range(K)], axis=1).astype(np.float16)
    idn2 = np.concatenate([eye * pi2[k] for k in range(K)], axis=1).astype(np.float16)
    xTc = np.ascontiguousarray(x.T)
    maskT = _dropout_mask_T()                                  # [64, 8192]

    if "prog" not in _prog_cache:
        _prog_cache["prog"] = _build_program()
    nc = _prog_cache["prog"]

    in_maps = []
    for d in range(NCORES):
        r0, r1 = d * R, (d + 1) * R
        in_maps.append({
            "adjb": np.ascontiguousarray(adj[:, r0:r1, :]),
            "adjc": np.ascontiguousarray(adj[:, :, r0:r1]),
            "xT": xTc,
            "w1d": W1,
            "idn1d": idn1,
            "idn2d": idn2,
            "w2d": W2,
            "maskTb": np.ascontiguousarray(maskT[:, r0:r1]),
            "b1c": b1.reshape(F_HID, 1),
        })

    res = run_bass_kernel_spmd(nc, in_maps, core_ids=list(range(NCORES)),
                               trace=trace)
    acc = np.zeros((F_OUT, N), dtype=np.float64)
    for d in range(NCORES):
        acc += res.results[d]["outT"]
    out = acc.T.astype(np.float32) + b2[None, :]
    return out, res


def kernel(**inputs) -> np.ndarray:
    out, _ = _run(inputs, trace=False)
    return out
